# revision 27
# baseline (speedup 1.0000x reference)
"""Trainium2 Bass kernel for ContPeepholeLSTMFunc_Delay.

Strategy (pure data parallel, batch sharded 8 ways; feature-major on-chip
layout [feature_chunk(128 part), batch(free)] via host transposes):

  - Matmuls run as fp8e4m3 DoubleRow instructions (2 k-chunks per inst,
    0.5 cyc/row) with mixed-precision compensation chosen per weight group
    from an error ablation against the 2e-2 tolerance:
      A  (1 product,  a8@W8):        dh.*, c.Pi, c.Pf, dc_dt@Po
      B  (2 products, acts hi+lo):   x.Wf, h.Uf
      B' (2 products, weights hi+lo): c_t@Po
      C  (3 products, both hi+lo):   x/h/cg @ Wi/Wc/Wo, cg.Wf
    All weights are stored at scale 16 (fp8-friendly); the 1/16 descale is
    folded into the ACT bias/scale and affine_mul_reduce s0/s1 operands.
  - Elementwise is restructured as
      dc_dt = f*((1-f)*pb*c + dc) + i*((1-i)*pa*c~ + (1-c~^2)*pd)
      dh    = o*((1-o)*T*pC + (1-T^2)*dc_dt)
    and balanced across ACT/DVE/Pool with bf16 tensor_tensor ops (DVE 2x).
  - Software pipeline: PE order is ph1(b0), ph1(b1), ph2(b0), ph1(b2),
    ph2(b1), ... so PE never waits on the elementwise tail of a block.
"""

import numpy as np
import ml_dtypes

import concourse.bacc as bacc
import concourse.mybir as mybir
import concourse.tile as tile
from concourse.bass_utils import run_bass_kernel_spmd

B, I, H = 32768, 256, 512
NCORES = 8
BC = B // NCORES            # 4096 rows per core
NBLK = 8                    # batch blocks per core
NB = BC // NBLK             # 512 batch columns per block
KI = I // 128               # 2 k-chunks for x-side contraction
KH = H // 128               # 4 k-chunks for h-side contraction
S = 16.0                    # fp8 weight scale
INV_S = 1.0 / S

F32 = mybir.dt.float32
BF16 = mybir.dt.bfloat16
FP8 = mybir.dt.float8e4
AF = mybir.ActivationFunctionType
OP = mybir.AluOpType
PM = mybir.MatmulPerfMode
BF16_NP = ml_dtypes.bfloat16
FP8_NP = ml_dtypes.float8_e4m3

# packed fp8 activation channel layout (32 chunks of [128, NB]):
#   x_hi 0:2 | h_hi 2:6 | c8 6:10 | x_lo 10:12 | h_lo 12:16 | x_16 16:18
#   h_16 18:22 | cg_hi 22:24 | cg_lo 24:26 | cg_16 26:28 | dh8 28:32
CH = {
    "x_hi": 0, "h_hi": 2, "c8": 6, "x_lo": 10, "h_lo": 12, "x_16": 16,
    "h_16": 18, "cg_hi": 22, "cg_lo": 24, "cg_16": 26, "dh8": 28,
}
NPK = 32


def build_nc(compile=True):
    nc = bacc.Bacc(None, target_bir_lowering=False)

    pk_d = nc.dram_tensor("pk8", [NBLK, 128, NPK, NB], FP8, kind="ExternalInput")
    pb_d = nc.dram_tensor("pb16", [NBLK, 128, 2 * KH, NB], BF16, kind="ExternalInput")

    # all weights packed (chunk-major) into two dram tensors: the zi set
    # (loaded first, gates startup) and the rest.
    # wpackA chunks: wi3[3*KI]=0..6, ui3[3*KH]=6..18, pi8=18..22
    # wpackB: wf3=0, uf3=6, pf8=18, wc3=22, uc3=28, upi=40, upf=44,
    #         wo3=48, uo3=54, po_m=66  (70 chunks)
    NWA, NWB = 22, 70
    wa_d = nc.dram_tensor("wpackA", [128, NWA, H], FP8, kind="ExternalInput")
    wb_d = nc.dram_tensor("wpackB", [128, NWB, H], FP8, kind="ExternalInput")
    b_d = nc.dram_tensor("biases", [128, 4 * KH], F32, kind="ExternalInput")

    out_d = nc.dram_tensor("outT", [NBLK, 128, KH, NB], BF16, kind="ExternalOutput")

    # (tensor, base, K) address map; 3-variant entries use off=base+j*K+2t
    WOFF = {
        "wi3": ("A", 0, KI), "ui3": ("A", 6, KH), "pi8": ("A", 18, KH),
        "wf3": ("B", 0, KI), "uf3": ("B", 6, KH), "pf8": ("B", 18, KH),
        "wc3": ("B", 22, KI), "uc3": ("B", 28, KH),
        "upi": ("B", 40, KH), "upf": ("B", 44, KH),
        "wo3": ("B", 48, KI), "uo3": ("B", 54, KH),
        "po_m": ("B", 66, KH),
    }

    with tile.TileContext(nc) as tc:
        with (
            tc.tile_pool(name="wpool", bufs=1) as wp,
            tc.tile_pool(name="inp", bufs=2) as inp,
            tc.tile_pool(name="blk", bufs=1) as blkp,
            tc.tile_pool(name="scr", bufs=2) as scr,
            tc.tile_pool(name="acc", bufs=8) as accp,
            tc.tile_pool(name="psum", bufs=8, space="PSUM") as pp,
        ):
            b_sb = wp.tile([128, 4 * KH], F32, tag="biases", name="b_sb")
            nc.sync.dma_start(b_sb[:], b_d[:])
            Bs = {
                "bi": b_sb[:, 0 * KH: 1 * KH],
                "bf": b_sb[:, 1 * KH: 2 * KH],
                "bc": b_sb[:, 2 * KH: 3 * KH],
                "bo": b_sb[:, 3 * KH: 4 * KH],
            }
            wa_sb = wp.tile([128, NWA, H], FP8, tag="wa", name="wa_sb")
            nc.sync.dma_start(wa_sb[:, 0:6], wa_d[:, 0:6])    # wi3 first
            nc.sync.dma_start(wa_sb[:, 6:NWA], wa_d[:, 6:NWA])
            WSB = {"A": wa_sb}

            def wpair(name, j, t):
                which, base, K = WOFF[name]
                off = base + (0 if j is None else j * K) + 2 * t
                return (WSB[which], off)

            def dr_seq(ps, pairs, mo):
                """DoubleRow accumulation; pairs = (rhs, (wsb, off))."""
                ms = slice(mo * 128, (mo + 1) * 128)
                n = len(pairs)
                for idx, (rhs, (wsb, off)) in enumerate(pairs):
                    nc.tensor.matmul(
                        ps[:], wsb[:, off: off + 2, ms], rhs,
                        start=(idx == 0), stop=(idx == n - 1),
                        perf_mode=PM.DoubleRow,
                    )

            def cpairs(pk, base, nch):
                return [pk[:, base + 2 * t: base + 2 * t + 2, :]
                        for t in range(nch // 2)]

            def cscheme(pk, wname, xkey, hname=None, hkey=None, nvar=3):
                out = []
                for j, part in ((0, "hi"), (1, "lo"), (2, "16"))[:nvar]:
                    out += [(r, wpair(wname, j, t)) for t, r in
                            enumerate(cpairs(pk, CH[f"{xkey}_{part}"], KI))]
                    if hname is not None:
                        out += [(r, wpair(hname, j, t)) for t, r in
                                enumerate(cpairs(pk, CH[f"{hkey}_{part}"], KH))]
                return out

            def apairs(pk, key, wname, nch=KH):
                return [(r, wpair(wname, None, t)) for t, r in
                        enumerate(cpairs(pk, CH[key], nch))]

            prev = None

            def emit_ph2_mo(pv, mo):
                """Phase 2 for one h-chunk of a completed block."""
                (pk, ct8, dc8, T, o_blk, t1_blk, m3, nb) = pv
                zo = pp.tile([128, NB], F32, tag="zo", bufs=1, name="zo")
                dr_seq(zo, (
                    cscheme(pk, "wo3", "x", "uo3", "h")
                    + [(ct8[:, 2 * t: 2 * t + 2, :], wpair("po_m", None, t)) for t in range(KH // 2)]
                ), mo)
                pC = pp.tile([128, NB], F32, tag="pC", bufs=1, name="pC")
                dr_seq(pC, (
                    cscheme(pk, "wo3", "cg")
                    + [(r, wpair("uo3", 0, t)) for t, r in enumerate(cpairs(pk, CH["dh8"], KH))]
                    + [(dc8[:, 2 * t: 2 * t + 2, :], wpair("po_m", None, t)) for t in range(KH // 2)]
                ), mo)

                nc.scalar.activation(o_blk[:, mo, :], zo[:], AF.Sigmoid,
                                     bias=Bs["bo"][:, mo: mo + 1], scale=INV_S)
                u3 = blkp.tile([128, NB], BF16, tag="u3", bufs=2, name="u3")
                a3 = accp.tile([128, 1], F32, tag="acc", name="a3")
                nc.vector.affine_mul_reduce(u3[:], a3[:], o_blk[:, mo, :],
                                            pC[:], -INV_S, INV_S)
                nc.vector.tensor_mul(t1_blk[:, mo, :], u3[:], T[:, mo, :])

            def emit_ph2_tail(pv, last=False):
                (pk, ct8, dc8, T, o_blk, t1_blk, m3, nb) = pv
                s3 = blkp.tile([128, KH, NB], BF16, tag="s3", name="s3")
                nc.vector.tensor_add(s3[:], t1_blk[:], m3[:])
                ob = blkp.tile([128, KH, NB], BF16, tag="dcdt", name="ob")
                (nc.vector if last else nc.gpsimd).tensor_mul(
                    ob[:], o_blk[:], s3[:])
                nc.sync.dma_start(out_d[nb], ob[:])

            def stile(tag):
                return scr.tile([128, NB], BF16, tag=tag, name=tag)

            for nb in range(NBLK):
                pk = inp.tile([128, NPK, NB], FP8, tag="pk", bufs=3, name="pk")
                if nb == 0:
                    # stage so zi's first products can start asap
                    nc.sync.dma_start(pk[:, 0:6], pk_d[nb, :, 0:6])
                    nc.sync.dma_start(pk[:, 6:22], pk_d[nb, :, 6:22])
                else:
                    nc.sync.dma_start(pk[:, 0:22], pk_d[nb, :, 0:22])
                nc.sync.dma_start(pk[:, 22:NPK], pk_d[nb, :, 22:NPK])
                pbf = inp.tile([128, 2 * KH, NB], BF16, tag="pbf", name="pbf")
                nc.sync.dma_start(pbf[:], pb_d[nb])
                c_sb = pbf[:, 0:KH]
                dc_sb = pbf[:, KH: 2 * KH]

                if nb == 0:
                    wb_sb = wp.tile([128, NWB, H], FP8, tag="wb", name="wb_sb")
                    # zf set | zc set + upi/upf; the zo set loads after block
                    # 1's inputs (first needed mid-block-1)
                    nc.sync.dma_start(wb_sb[:, 0:22], wb_d[:, 0:22])
                    nc.sync.dma_start(wb_sb[:, 22:48], wb_d[:, 22:48])
                    WSB["B"] = wb_sb
                if nb == 1:
                    nc.sync.dma_start(WSB["B"][:, 48:NWB], wb_d[:, 48:NWB])

                dcdt = blkp.tile([128, KH, NB], BF16, tag="dcdt", name="dcdt")
                T = blkp.tile([128, KH, NB], BF16, tag="T", bufs=2, name="T")
                ct8 = blkp.tile([128, KH, NB], FP8, tag="ct8", bufs=2, name="ct8")
                dc8 = blkp.tile([128, KH, NB], FP8, tag="dc8", bufs=2, name="dc8")
                o_blk = blkp.tile([128, KH, NB], BF16, tag="o", bufs=2, name="o_blk")
                t1_blk = blkp.tile([128, KH, NB], BF16, tag="t1", bufs=2, name="t1_blk")
                m3 = blkp.tile([128, KH, NB], BF16, tag="m3", bufs=2, name="m3")

                for mo in range(KH):
                    # group order zi,pa,zf,pb,zc,pd: each bank's drain starts
                    # right after its producer stops, maximizing recycle slack
                    zi = pp.tile([128, NB], F32, tag="zi", bufs=1, name="zi")
                    dr_seq(zi, (
                        cscheme(pk, "wi3", "x", "ui3", "h")
                        + apairs(pk, "c8", "pi8")
                    ), mo)
                    i_t = stile("i")
                    nc.scalar.activation(i_t[:], zi[:], AF.Sigmoid,
                                         bias=Bs["bi"][:, mo: mo + 1], scale=INV_S)

                    zf = pp.tile([128, NB], F32, tag="zf", bufs=1, name="zf")
                    zf_pairs = [(r, wpair("wf3", 0, t)) for t, r in
                                enumerate(cpairs(pk, CH["x_hi"], KI))]
                    zf_pairs += [(r, wpair("uf3", 0, t)) for t, r in
                                 enumerate(cpairs(pk, CH["h_hi"], KH))]
                    zf_pairs += [(r, wpair("wf3", 1, t)) for t, r in
                                 enumerate(cpairs(pk, CH["x_lo"], KI))]
                    zf_pairs += apairs(pk, "c8", "pf8")
                    dr_seq(zf, zf_pairs, mo)
                    f_t = stile("f")
                    nc.scalar.activation(f_t[:], zf[:], AF.Sigmoid,
                                         bias=Bs["bf"][:, mo: mo + 1], scale=INV_S)

                    zc = pp.tile([128, NB], F32, tag="zc", bufs=1, name="zc")
                    dr_seq(zc, cscheme(pk, "wc3", "x", "uc3", "h"), mo)
                    cti = stile("cti")
                    nc.scalar.activation(cti[:], zc[:], AF.Tanh,
                                         bias=Bs["bc"][:, mo: mo + 1], scale=INV_S)
                    sq = stile("sq")
                    nc.scalar.activation(sq[:], cti[:], AF.Square)

                    pa = pp.tile([128, NB], F32, tag="pa", bufs=1, name="pa")
                    dr_seq(pa, (cscheme(pk, "wi3", "cg")
                                + apairs(pk, "dh8", "upi")), mo)
                    u2 = stile("u2")
                    a2 = accp.tile([128, 1], F32, tag="acc", name="a2")
                    nc.vector.affine_mul_reduce(u2[:], a2[:], i_t[:], pa[:],
                                                -INV_S, INV_S)

                    pb = pp.tile([128, NB], F32, tag="pb", bufs=1, name="pb")
                    dr_seq(pb, (cscheme(pk, "wf3", "cg")
                                + apairs(pk, "dh8", "upf")), mo)
                    u1 = stile("u1")
                    a1 = accp.tile([128, 1], F32, tag="acc", name="a1")
                    nc.vector.affine_mul_reduce(u1[:], a1[:], f_t[:], pb[:],
                                                -INV_S, INV_S)

                    pd = pp.tile([128, NB], F32, tag="pd", bufs=1, name="pd")
                    dr_seq(pd, (
                        cscheme(pk, "wc3", "cg")
                        + [(r, wpair("uc3", 0, t)) for t, r in enumerate(cpairs(pk, CH["dh8"], KH))]
                    ), mo)
                    m2 = stile("m2")
                    am = accp.tile([128, 1], F32, tag="acc", name="am")
                    nc.vector.affine_mul_reduce(m2[:], am[:], sq[:], pd[:],
                                                -INV_S, INV_S)

                    # previous block's phase 2 (its o/u3/t1 drains free zo/pC)
                    if prev is not None:
                        emit_ph2_mo(prev, mo)

                    v2 = stile("v2")
                    nc.vector.tensor_mul(v2[:], u2[:], cti[:])
                    v1 = stile("v1")
                    nc.vector.tensor_mul(v1[:], u1[:], c_sb[:, mo, :])
                    tfc = stile("tfc")
                    nc.vector.tensor_mul(tfc[:], f_t[:], c_sb[:, mo, :])
                    mic = stile("mic")
                    nc.vector.tensor_mul(mic[:], i_t[:], cti[:])
                    ct = stile("ct")
                    nc.vector.tensor_add(ct[:], tfc[:], mic[:])
                    nc.scalar.activation(T[:, mo, :], ct[:], AF.Tanh)
                    nc.scalar.activation(ct8[:, mo, :], ct[:], AF.Copy)

                    w1 = stile("w1")
                    nc.gpsimd.tensor_add(w1[:], v1[:], dc_sb[:, mo, :])
                    r1 = stile("r1")
                    nc.vector.tensor_mul(r1[:], f_t[:], w1[:])
                    w2 = stile("w2")
                    nc.gpsimd.tensor_add(w2[:], v2[:], m2[:])
                    r2 = stile("r2")
                    nc.vector.tensor_mul(r2[:], i_t[:], w2[:])
                    nc.vector.tensor_add(dcdt[:, mo, :], r1[:], r2[:])
                    nc.scalar.activation(dc8[:, mo, :], dcdt[:, mo, :], AF.Copy)

                    sqt = stile("sqt")
                    nc.scalar.activation(sqt[:], T[:, mo, :], AF.Square)
                    sqtm = stile("sqtm")
                    nc.scalar.activation(sqtm[:], sqt[:], AF.Copy, bias=1.0,
                                         scale=-1.0)
                    nc.gpsimd.tensor_mul(m3[:, mo, :], sqtm[:], dcdt[:, mo, :])

                if prev is not None:
                    emit_ph2_tail(prev)
                prev = (pk, ct8, dc8, T, o_blk, t1_blk, m3, nb)

            for mo in range(KH):
                emit_ph2_mo(prev, mo)
            emit_ph2_tail(prev, last=True)

    if compile:
        nc.compile()
    return nc


_NC_CACHE = None


def _get_nc():
    global _NC_CACHE
    if _NC_CACHE is None:
        _NC_CACHE = build_nc()
    return _NC_CACHE


def _featmajor(a, K):
    """[BC, K*128] row-major -> [NBLK, 128, K, NB] feature-major fp32."""
    return a.reshape(NBLK, NB, K, 128).transpose(0, 3, 2, 1)


def _fp8(a):
    return np.asarray(a, np.float32).astype(FP8_NP)


def _prep_w_base(w):
    """W [H_out, K_in] -> [128, K_in//128, H_out] fp32 (lhsT layout)."""
    wt = np.asarray(w, np.float32).T
    k = wt.shape[0] // 128
    return np.ascontiguousarray(wt.reshape(k, 128, wt.shape[1]).transpose(1, 0, 2))


def _prep_w3(w):
    """-> [128, 3, K, H] fp8: main=q(16W), half=q(W), lo=q(16*(16W-main))."""
    base = _prep_w_base(w)
    m = (S * base).astype(FP8_NP)
    half = base.astype(FP8_NP)
    lo = (16.0 * (S * base - m.astype(np.float32))).astype(FP8_NP)
    return np.ascontiguousarray(np.stack([m, half, lo], axis=1))


def _prep_w1(w):
    """-> [128, K, H] fp8 at scale 16."""
    return np.ascontiguousarray((S * _prep_w_base(w)).astype(FP8_NP))


def _prep_w_lo(w):
    """-> [128, K, H] fp8 residual 16*(16W - q(16W))."""
    base = _prep_w_base(w)
    m = (S * base).astype(FP8_NP)
    return np.ascontiguousarray((16.0 * (S * base - m.astype(np.float32))).astype(FP8_NP))


def _prep_b(b):
    return np.ascontiguousarray(np.asarray(b, np.float32).reshape(KH, 128).T)


def _run(inputs, trace=False):
    nc = _get_nc()

    g = lambda k: np.asarray(inputs[k], np.float32)

    def w3flat(w):
        """[128, 3, K, H] -> [128, 3K, H]"""
        a = _prep_w3(w)
        return a.reshape(128, -1, H)

    wpackA = np.ascontiguousarray(np.concatenate([
        w3flat(g("Wi")), w3flat(g("Ui")), _prep_w1(g("Pi")),
    ], axis=1))
    wpackB = np.ascontiguousarray(np.concatenate([
        w3flat(g("Wf")), w3flat(g("Uf")), _prep_w1(g("Pf")),
        w3flat(g("Wc")), w3flat(g("Uc")),
        _prep_w1(g("Ui") + g("Pi")), _prep_w1(g("Uf") + g("Pf")),
        w3flat(g("Wo")), w3flat(g("Uo")),
        _prep_w1(g("Po")),
    ], axis=1))
    wmap = {
        "wpackA": wpackA,
        "wpackB": wpackB,
        "biases": np.ascontiguousarray(np.concatenate([
            _prep_b(g("bUi") + g("bPi")),
            _prep_b(g("bUf") + g("bPf")),
            _prep_b(g("bUc")),
            _prep_b(g("bUo") + g("bPo")),
        ], axis=1)),
    }

    x = g("x"); cg = g("control_grad"); h = g("h_past")
    dh = g("dhpast_dt"); c = g("c_past"); dc = g("dcpast_dt")

    in_maps = []
    for core in range(NCORES):
        sl = slice(core * BC, (core + 1) * BC)
        m = dict(wmap)
        pk = np.empty([NBLK, 128, NPK, NB], FP8_NP)

        def put3(a, K, hi_c, lo_c, s16_c):
            fm = _featmajor(a, K)
            hi = fm.astype(FP8_NP)
            pk[:, :, hi_c: hi_c + K] = hi
            pk[:, :, lo_c: lo_c + K] = (16.0 * (fm - hi.astype(np.float32))).astype(FP8_NP)
            pk[:, :, s16_c: s16_c + K] = (fm * INV_S).astype(FP8_NP)

        put3(x[sl], KI, CH["x_hi"], CH["x_lo"], CH["x_16"])
        put3(h[sl], KH, CH["h_hi"], CH["h_lo"], CH["h_16"])
        put3(cg[sl], KI, CH["cg_hi"], CH["cg_lo"], CH["cg_16"])
        cfm = _featmajor(c[sl], KH)
        pk[:, :, CH["c8"]: CH["c8"] + KH] = cfm.astype(FP8_NP)
        pk[:, :, CH["dh8"]: CH["dh8"] + KH] = _featmajor(dh[sl], KH).astype(FP8_NP)
        m["pk8"] = pk

        pbf = np.empty([NBLK, 128, 2 * KH, NB], BF16_NP)
        pbf[:, :, 0:KH] = cfm
        pbf[:, :, KH: 2 * KH] = _featmajor(dc[sl], KH)
        m["pb16"] = pbf
        in_maps.append(m)

    try:
        res = run_bass_kernel_spmd(nc, in_maps, core_ids=list(range(NCORES)), trace=trace)
    except ModuleNotFoundError:
        if not trace:
            raise
        res = run_bass_kernel_spmd(nc, in_maps, core_ids=list(range(NCORES)), trace=False)

    outs = []
    for core in range(NCORES):
        o = res.results[core]["outT"]  # [NBLK, 128, KH, NB] bf16
        o = np.asarray(o, np.float32)
        # out[p, mo, col] is feature mo*128+p of batch row nb*NB+col
        o = o.transpose(0, 3, 2, 1).reshape(BC, H)
        outs.append(o)
    full = np.ascontiguousarray(np.concatenate(outs, axis=0), dtype=np.float32)
    return full, res


def kernel(**inputs):
    return _run(inputs, trace=False)[0]
